# revision 10
# baseline (speedup 1.0000x reference)
"""Trainium2 Bass kernel for a causal multi-head attention block.

Reference computation (per nn_Attend):
    q = (x @ Wq + bq), k = (x @ Wk + bk), v = (x @ Wv + bv)   (per-head split)
    att = softmax(causal(q k^T / sqrt(hd)))
    y = (att v) @ Wo + bo

Sharding: tensor-parallel over heads across 8 NeuronCores. Core c gets
heads [2c, 2c+1]: column shards of Wq/Wk/Wv (+bias shards) and the matching
row shard of Wo. Every core computes a full-size partial output
yt_c = (att_out_c @ Wo_c)^T; the host sums the 8 partials, adds bo, and
transposes back.

On-chip layout is "transposed": activations live as [feature, token] so that
every matmul's contraction dim sits on SBUF partitions:
  QT/KT/VT = W^T @ x^T  (lhsT = W in natural [in,out] layout, rhs = x^T)
  scoresT[k, q] = (K^T)^T-slice @ Q^T      (contraction over head_dim)
  att_outT[hd, q] = sum_kt V[kt]^T-as-lhsT @ expT[kt]  (contraction over keys)
  yT[out, q] = Wo-slice-as-lhsT @ att_outT (contraction over per-core head dims)
V is needed in natural [token, hd] layout as lhsT; it is produced by PE
transposes of VT with an extra all-ones column so that each att_outT
accumulation also yields the softmax row-sums for free. Softmax is unmasked
exp (scores are bounded, no max subtraction needed); the causal mask is
"compute only the valid triangle" + a 0/1 upper-triangular mask multiply on
diagonal 128x128 blocks. Normalization divides att_outT columns by the
broadcast row-sums before the output projection.

All matmuls run in float16 (full PE rate, fast weight load).
"""

import os
from contextlib import ExitStack
from dataclasses import dataclass

import numpy as np

import concourse.bass as bass
import concourse.tile as tile
from concourse import bacc, mybir
from concourse.masks import make_identity, make_upper_triangular

F32 = mybir.dt.float32
F32R = mybir.dt.float32r
F16 = mybir.dt.float16
P = 128  # SBUF partitions


@dataclass(frozen=True)
class Cfg:
    B: int = 2
    S: int = 2048
    D: int = 1024
    H: int = 16
    NCORES: int = 8
    RC: int = 512        # row chunk for QKV projection streaming
    PW: int = 1024       # psum piece width for scoresT (2 banks)

    @property
    def HPC(self):  # heads per core
        return self.H // self.NCORES

    @property
    def hd(self):  # head dim
        return self.D // self.H

    @property
    def COLS(self):  # per-core projection output columns
        return self.HPC * self.hd

    @property
    def KT(self):  # contraction tiles for projections
        return self.D // P

    @property
    def ROWS(self):
        return self.B * self.S

    @property
    def SKT(self):  # key tiles per batch
        return self.S // P

    @property
    def QCW(self):  # q chunk width for att_out accumulation
        return min(512, self.S)

    @property
    def NQC(self):
        return self.S // self.QCW

    @property
    def KPC(self):  # key tiles per q-chunk
        return self.QCW // P

    @property
    def VTC(self):  # v-tile columns per head (head_dim + ones column)
        return self.hd + 1


def _exp_offsets(cfg: Cfg):
    """Free-dim offsets of each key-tile's strip in the exp buffer."""
    offs, total = [], 0
    for kt in range(cfg.SKT):
        offs.append(total)
        total += cfg.S - P * kt
    return offs, total


def emit_attention(tc: tile.TileContext, io: dict, cfg: Cfg):
    nc = tc.nc
    xt, wq, wk, wv, wo = io["xt"], io["wq"], io["wk"], io["wv"], io["wo"]
    bq, bk, bv, yt = io["bq"], io["bk"], io["bv"], io["yt"]

    COLS, KT, RC, ROWS = cfg.COLS, cfg.KT, cfg.RC, cfg.ROWS
    NRC = ROWS // RC
    S, SKT, B, hd, HPC = cfg.S, cfg.SKT, cfg.B, cfg.hd, cfg.HPC
    QCW, NQC, KPC, VTC, PW = cfg.QCW, cfg.NQC, cfg.KPC, cfg.VTC, cfg.PW
    NT = ROWS // P  # v row-tiles
    offs, expw = _exp_offsets(cfg)

    with ExitStack() as ctx:
        wpool = ctx.enter_context(tc.tile_pool(name="wpool", bufs=1))
        qkpool = ctx.enter_context(tc.tile_pool(name="qkpool", bufs=1))
        vpool = ctx.enter_context(tc.tile_pool(name="vpool", bufs=1))

        # ---- constants ----
        wq_sb = wpool.tile([P, KT, COLS], F16, tag="wq")
        wk_sb = wpool.tile([P, KT, COLS], F16, tag="wk")
        wv_sb = wpool.tile([P, KT, COLS], F16, tag="wv")
        wo_sb = wpool.tile([COLS, cfg.D], F16, tag="wo")
        bq_sb = wpool.tile([COLS, 1], F32, tag="bq")
        bk_sb = wpool.tile([COLS, 1], F32, tag="bk")
        bv_sb = wpool.tile([COLS, 1], F32, tag="bv")
        ident = wpool.tile([P, P], F16, tag="ident")
        umask = wpool.tile([P, P], F16, tag="umask")

        for w_dram, w_sb in ((wq, wq_sb), (wk, wk_sb), (wv, wv_sb)):
            nc.sync.dma_start(
                out=w_sb, in_=w_dram.rearrange("(kt p) c -> p kt c", p=P)
            )
        nc.sync.dma_start(out=wo_sb, in_=wo)
        for b_dram, b_sb in ((bq, bq_sb), (bk, bk_sb), (bv, bv_sb)):
            nc.sync.dma_start(out=b_sb, in_=b_dram.rearrange("(c one) -> c one", one=1))
        make_identity(nc, ident)
        make_upper_triangular(nc, umask, val=1.0, diag=True)

        # ---- phase A: projections QT/KT/VT + V transpose ----
        qt_sb = qkpool.tile([COLS, ROWS], F16, tag="qt")
        kt_sb = qkpool.tile([COLS, ROWS], F16, tag="kt")
        v_sb = vpool.tile([P, NT, HPC * VTC], F16, tag="v")
        # ones columns for the row-sum trick
        nc.vector.memset(v_sb[:, :, hd :: VTC], 1.0)

        with (
            tc.tile_pool(name="xpool", bufs=2) as xpool,
            tc.tile_pool(name="vtpool", bufs=2) as vtpool,
            tc.tile_pool(name="psA", bufs=2, space="PSUM") as psA,
            tc.tile_pool(name="psT", bufs=2, space="PSUM") as psT,
        ):
            for rc in range(NRC):
                xt_t = xpool.tile([P, KT, RC], F16, tag="xt")
                nc.sync.dma_start(
                    out=xt_t,
                    in_=xt[:, rc * RC : (rc + 1) * RC].rearrange(
                        "(kt p) n -> p kt n", p=P
                    ),
                )
                vt_t = vtpool.tile([COLS, RC], F16, tag="vt")
                for w_sb, b_sb, dest in (
                    (wq_sb, bq_sb, qt_sb),
                    (wk_sb, bk_sb, kt_sb),
                    (wv_sb, bv_sb, vt_t),
                ):
                    ps = psA.tile([COLS, RC], F32, tag="proj")
                    for kt in range(KT):
                        nc.tensor.matmul(
                            ps,
                            w_sb[:, kt, :],
                            xt_t[:, kt, :],
                            start=(kt == 0),
                            stop=(kt == KT - 1),
                        )
                    if dest is vt_t:
                        out_ap = vt_t
                    else:
                        out_ap = dest[:, rc * RC : (rc + 1) * RC]
                    nc.scalar.activation(
                        out=out_ap, in_=ps,
                        func=mybir.ActivationFunctionType.Identity,
                        bias=b_sb, scale=1.0,
                    )
                # transpose VT chunk into natural-layout v tiles
                for j in range(RC // P):
                    t = rc * (RC // P) + j
                    pst = psT.tile([P, COLS], F16, tag="vtr")
                    nc.tensor.matmul(
                        pst,
                        vt_t[:, j * P : (j + 1) * P],
                        ident[:COLS, :COLS],
                        is_transpose=True,
                    )
                    nc.vector.tensor_copy(
                        out=v_sb[:, t, :].rearrange("p (h c) -> p h c", h=HPC)[
                            :, :, 0:hd
                        ],
                        in_=pst.rearrange("p (h c) -> p h c", h=HPC),
                    )

        # ---- phases B/C/D per (batch, head) ----
        with (
            tc.tile_pool(name="expool", bufs=1) as expool,
            tc.tile_pool(name="apool", bufs=2) as apool,
            tc.tile_pool(name="spool", bufs=4) as spool,
            tc.tile_pool(name="opool", bufs=3) as opool,
            tc.tile_pool(name="psB", bufs=2, space="PSUM") as psB,
            tc.tile_pool(name="psC", bufs=4, space="PSUM") as psC,
        ):
            att_tiles = {}
            for b in range(B):
                for h in range(HPC):
                    if h == 0:
                        att_tiles[b] = apool.tile(
                            [COLS, S], F16, tag="att", name=f"att{b}"
                        )
                    att_sb = att_tiles[b]
                    exp_sb = expool.tile([P, expw], F16, tag="exp")
                    hp = h * hd  # partition offset of this head in QT/KT

                    # -- B: scoresT -> exp --
                    for kt in range(SKT):
                        w = S - P * kt
                        off = offs[kt]
                        q0 = b * S + P * kt  # global q col of strip start
                        for p0 in range(0, w, PW):
                            pw = min(PW, w - p0)
                            ps = psB.tile([P, PW], F32, tag="sc")
                            for s0 in range(0, pw, 512):
                                sw = min(512, pw - s0)
                                nc.tensor.matmul(
                                    ps[:, s0 : s0 + sw],
                                    kt_sb[
                                        hp : hp + hd, b * S + P * kt : b * S + P * (kt + 1)
                                    ],
                                    qt_sb[hp : hp + hd, q0 + p0 + s0 : q0 + p0 + s0 + sw],
                                    start=True,
                                    stop=True,
                                )
                            nc.scalar.activation(
                                out=exp_sb[:, off + p0 : off + p0 + pw],
                                in_=ps[:, 0:pw],
                                func=mybir.ActivationFunctionType.Exp,
                            )
                        # causal mask on the diagonal block
                        nc.vector.tensor_tensor(
                            out=exp_sb[:, off : off + P],
                            in0=exp_sb[:, off : off + P],
                            in1=umask,
                            op=mybir.AluOpType.mult,
                        )

                    # -- C: att_outT accumulation + normalize --
                    for c in range(NQC):
                        pa = psC.tile([VTC, QCW], F32, tag="acc")
                        kt_hi = min(SKT, KPC * (c + 1))
                        first = True
                        for kt in range(kt_hi):
                            lo = max(QCW * c, P * kt)
                            hi = QCW * (c + 1)
                            n = hi - lo
                            if n <= 0:
                                continue
                            nc.tensor.matmul(
                                pa[:, lo - QCW * c : lo - QCW * c + n],
                                v_sb[:, b * SKT + kt, h * VTC : (h + 1) * VTC],
                                exp_sb[:, offs[kt] + lo - P * kt : offs[kt] + lo - P * kt + n],
                                start=first,
                                stop=(kt == kt_hi - 1),
                            )
                            first = False
                        recip = spool.tile([1, QCW], F32, tag="recip")
                        nc.vector.reciprocal_approx_fast(out=recip, in_=pa[hd : hd + 1, :])
                        rbc = spool.tile([hd, QCW], F32, tag="rbc")
                        nc.gpsimd.partition_broadcast(rbc, recip[0:1, :])
                        nc.vector.tensor_tensor(
                            out=att_sb[hp : hp + hd, QCW * c : QCW * (c + 1)],
                            in0=pa[0:hd, :],
                            in1=rbc,
                            op=mybir.AluOpType.mult,
                        )

                # -- D: output projection for batch b --
                for m in range(cfg.D // P):
                    for c in range(NQC):
                        po = psC.tile([P, QCW], F32, tag="acc")
                        nc.tensor.matmul(
                            po,
                            wo_sb[:, m * P : (m + 1) * P],
                            att_sb[:, QCW * c : QCW * (c + 1)],
                            start=True,
                            stop=True,
                        )
                        o_sb = opool.tile([P, QCW], F32, tag="o")
                        nc.vector.tensor_copy(out=o_sb, in_=po)
                        nc.sync.dma_start(
                            out=yt[
                                m * P : (m + 1) * P, b * S + QCW * c : b * S + QCW * (c + 1)
                            ],
                            in_=o_sb,
                        )


def build_model(cfg: Cfg) -> bass.Bass:
    nc = bacc.Bacc(
        "TRN2", target_bir_lowering=False, debug=False, enable_asserts=False
    )
    io = {
        "xt": nc.dram_tensor("xt", [cfg.D, cfg.ROWS], F16, kind="ExternalInput").ap(),
        "wq": nc.dram_tensor("wq", [cfg.D, cfg.COLS], F16, kind="ExternalInput").ap(),
        "wk": nc.dram_tensor("wk", [cfg.D, cfg.COLS], F16, kind="ExternalInput").ap(),
        "wv": nc.dram_tensor("wv", [cfg.D, cfg.COLS], F16, kind="ExternalInput").ap(),
        "wo": nc.dram_tensor("wo", [cfg.COLS, cfg.D], F16, kind="ExternalInput").ap(),
        "bq": nc.dram_tensor("bq", [cfg.COLS], F32, kind="ExternalInput").ap(),
        "bk": nc.dram_tensor("bk", [cfg.COLS], F32, kind="ExternalInput").ap(),
        "bv": nc.dram_tensor("bv", [cfg.COLS], F32, kind="ExternalInput").ap(),
        "yt": nc.dram_tensor("yt", [cfg.D, cfg.ROWS], F32, kind="ExternalOutput").ap(),
    }
    with tile.TileContext(nc) as tc:
        emit_attention(tc, io, cfg)
    nc.finalize()
    return nc


def shard_inputs(cfg: Cfg, x, Wq, bq, Wk, bk, Wv, bv, Wo, bo):
    """Full inputs -> per-core in_maps (host side)."""
    scale = 1.0 / np.sqrt(np.float32(cfg.hd))
    xt = np.ascontiguousarray(
        np.asarray(x, dtype=np.float32).reshape(cfg.ROWS, cfg.D).T
    ).astype(np.float16)
    in_maps = []
    for c in range(cfg.NCORES):
        sl = slice(cfg.COLS * c, cfg.COLS * (c + 1))
        in_maps.append(
            {
                "xt": xt,
                "wq": np.ascontiguousarray(np.asarray(Wq)[:, sl] * scale).astype(np.float16),
                "bq": np.ascontiguousarray(np.asarray(bq)[sl] * scale),
                "wk": np.ascontiguousarray(np.asarray(Wk)[:, sl]).astype(np.float16),
                "bk": np.ascontiguousarray(np.asarray(bk)[sl]),
                "wv": np.ascontiguousarray(np.asarray(Wv)[:, sl]).astype(np.float16),
                "bv": np.ascontiguousarray(np.asarray(bv)[sl]),
                "wo": np.ascontiguousarray(np.asarray(Wo)[sl, :]).astype(np.float16),
            }
        )
    return in_maps


def unshard_output(cfg: Cfg, per_core_yt, bo):
    acc = per_core_yt[0].astype(np.float32)
    for yt_c in per_core_yt[1:]:
        acc = acc + yt_c
    y = acc.T + np.asarray(bo, dtype=np.float32)
    return np.ascontiguousarray(y.reshape(cfg.B, cfg.S, cfg.D)).astype(np.float32)


_MODEL = None


def _get_model(cfg: Cfg):
    global _MODEL
    if _MODEL is None:
        _MODEL = build_model(cfg)
    return _MODEL


def kernel(x, Wq, bq, Wk, bk, Wv, bv, Wo, bo, _trace=False):
    from concourse.bass_utils import run_bass_kernel_spmd

    cfg = Cfg()
    in_maps = shard_inputs(cfg, x, Wq, bq, Wk, bk, Wv, bv, Wo, bo)
    nc = _get_model(cfg)
    res = run_bass_kernel_spmd(
        nc, in_maps, core_ids=list(range(cfg.NCORES)), trace=_trace
    )
    y = unshard_output(cfg, [r["yt"] for r in res.results], bo)
    if _trace:
        return y, res
    return y


# revision 13
# speedup vs baseline: 1.1693x; 1.1693x over previous
"""Trainium2 Bass kernel for a causal multi-head attention block.

Reference computation (per nn_Attend):
    q = (x @ Wq + bq), k = (x @ Wk + bk), v = (x @ Wv + bv)   (per-head split)
    att = softmax(causal(q k^T / sqrt(hd)))
    y = (att v) @ Wo + bo

Sharding: tensor-parallel over heads across 8 NeuronCores. Core c gets
heads [2c, 2c+1]: column shards of Wq/Wk/Wv (+bias shards) and the matching
row shard of Wo. Every core computes a full-size partial output
yt_c = (att_out_c @ Wo_c)^T; the host sums the 8 partials, adds bo, and
transposes back.

On-chip layout is "transposed": activations live as [feature, token] so that
every matmul's contraction dim sits on SBUF partitions:
  QT/KT/VT = W^T @ x^T  (lhsT = W in natural [in,out] layout, rhs = x^T)
  scoresT[k, q] = (K^T)^T-slice @ Q^T      (contraction over head_dim)
  att_outT[hd, q] = sum_kt V[kt]^T-as-lhsT @ expT[kt]  (contraction over keys)
  yT[out, q] = Wo-slice-as-lhsT @ att_outT (contraction over per-core head dims)
V is needed in natural [token, hd] layout as lhsT; it is produced by PE
transposes of VT with an extra all-ones column so that each att_outT
accumulation also yields the softmax row-sums for free. Softmax is unmasked
exp (scores are bounded, no max subtraction needed); the causal mask is
"compute only the valid triangle" + a 0/1 upper-triangular mask multiply on
diagonal 128x128 blocks. Normalization divides att_outT columns by the
broadcast row-sums before the output projection.

All matmuls run in float16 (full PE rate, fast weight load).
"""

import os
from contextlib import ExitStack
from dataclasses import dataclass

import numpy as np

import concourse.bass as bass
import concourse.tile as tile
from concourse import bacc, mybir
from concourse.masks import make_identity, make_upper_triangular

F32 = mybir.dt.float32
F32R = mybir.dt.float32r
F16 = mybir.dt.float16
P = 128  # SBUF partitions


@dataclass(frozen=True)
class Cfg:
    B: int = 2
    S: int = 2048
    D: int = 1024
    H: int = 16
    NCORES: int = 8
    RC: int = 512        # row chunk for QKV projection streaming
    PW: int = 1024       # psum piece width for scoresT (2 banks)

    @property
    def HPC(self):  # heads per core
        return self.H // self.NCORES

    @property
    def hd(self):  # head dim
        return self.D // self.H

    @property
    def COLS(self):  # per-core projection output columns
        return self.HPC * self.hd

    @property
    def KT(self):  # contraction tiles for projections
        return self.D // P

    @property
    def ROWS(self):
        return self.B * self.S

    @property
    def SKT(self):  # key tiles per batch
        return self.S // P

    @property
    def QCW(self):  # q chunk width for att_out accumulation
        return min(512, self.S)

    @property
    def NQC(self):
        return self.S // self.QCW

    @property
    def KPC(self):  # key tiles per q-chunk
        return self.QCW // P

    @property
    def VTC(self):  # v-tile columns per head (head_dim + ones column)
        return self.hd + 1


def _exp_offsets(cfg: Cfg):
    """Free-dim offsets of each key-tile's strip in the exp buffer."""
    offs, total = [], 0
    for kt in range(cfg.SKT):
        offs.append(total)
        total += cfg.S - P * kt
    return offs, total


def emit_attention(tc: tile.TileContext, io: dict, cfg: Cfg):
    nc = tc.nc
    xt, wq, wk, wv, wo = io["xt"], io["wq"], io["wk"], io["wv"], io["wo"]
    bq, bk, bv, yt = io["bq"], io["bk"], io["bv"], io["yt"]

    COLS, KT, RC, ROWS = cfg.COLS, cfg.KT, cfg.RC, cfg.ROWS
    NRC = ROWS // RC
    S, SKT, B, hd, HPC = cfg.S, cfg.SKT, cfg.B, cfg.hd, cfg.HPC
    QCW, NQC, KPC, VTC, PW = cfg.QCW, cfg.NQC, cfg.KPC, cfg.VTC, cfg.PW
    NT = ROWS // P  # v row-tiles
    offs, expw = _exp_offsets(cfg)

    with ExitStack() as ctx:
        wpool = ctx.enter_context(tc.tile_pool(name="wpool", bufs=1))
        qkpool = ctx.enter_context(tc.tile_pool(name="qkpool", bufs=1))
        vpool = ctx.enter_context(tc.tile_pool(name="vpool", bufs=1))

        # ---- constants ----
        wq_sb = wpool.tile([P, KT, COLS], F16, tag="wq")
        wk_sb = wpool.tile([P, KT, COLS], F16, tag="wk")
        wv_sb = wpool.tile([P, KT, COLS], F16, tag="wv")
        wo_sb = wpool.tile([COLS, cfg.D], F16, tag="wo")
        bq_sb = wpool.tile([COLS, 1], F32, tag="bq")
        bk_sb = wpool.tile([COLS, 1], F32, tag="bk")
        bv_sb = wpool.tile([COLS, 1], F32, tag="bv")
        ident = wpool.tile([P, P], F16, tag="ident")
        umask = wpool.tile([P, P], F16, tag="umask")

        for w_dram, w_sb in ((wq, wq_sb), (wk, wk_sb), (wv, wv_sb)):
            nc.sync.dma_start(
                out=w_sb, in_=w_dram.rearrange("(kt p) c -> p kt c", p=P)
            )
        nc.sync.dma_start(out=wo_sb, in_=wo)
        for b_dram, b_sb in ((bq, bq_sb), (bk, bk_sb), (bv, bv_sb)):
            nc.sync.dma_start(out=b_sb, in_=b_dram.rearrange("(c one) -> c one", one=1))
        make_identity(nc, ident)
        make_upper_triangular(nc, umask, val=1.0, diag=True)

        # ---- phase A: projections QT/KT/VT + V transpose ----
        qt_sb = qkpool.tile([COLS, ROWS], F16, tag="qt")
        kt_sb = qkpool.tile([COLS, ROWS], F16, tag="kt")
        vt_sb = qkpool.tile([COLS, ROWS], F16, tag="vt")
        v_sb = vpool.tile([P, NT, HPC * VTC], F16, tag="v")
        # ones columns for the row-sum trick
        nc.vector.memset(v_sb[:, :, hd :: VTC], 1.0)

        HC = COLS // 2  # column-tile half

        with (
            tc.tile_pool(name="xpool", bufs=2) as xpool,
            tc.tile_pool(name="psA", bufs=2, space="PSUM") as psA,
            tc.tile_pool(name="psT", bufs=2, space="PSUM") as psT,
        ):
            for rc in range(NRC):
                xt_t = xpool.tile([P, KT, RC], F16, tag="xt")
                nc.sync.dma_start(
                    out=xt_t,
                    in_=xt[:, rc * RC : (rc + 1) * RC].rearrange(
                        "(kt p) n -> p kt n", p=P
                    ),
                )
                for wi, (w_sb, b_sb, dest) in enumerate((
                    (wq_sb, bq_sb, qt_sb),
                    (wk_sb, bk_sb, kt_sb),
                    (wv_sb, bv_sb, vt_sb),
                )):
                    ps = psA.tile([COLS, RC], F32, tag="proj", name=f"proj{rc}_{wi}")
                    # two column-tile chains (T0/T1) run concurrently on PE
                    for kt in range(KT):
                        for half in range(2):
                            nc.tensor.matmul(
                                ps[half * HC : (half + 1) * HC, :],
                                w_sb[:, kt, half * HC : (half + 1) * HC],
                                xt_t[:, kt, :],
                                start=(kt == 0),
                                stop=(kt == KT - 1),
                                skip_group_check=True,
                            )
                    out_ap = dest[:, rc * RC : (rc + 1) * RC]
                    if wi == 0:  # ACT eviction with bias
                        nc.scalar.activation(
                            out=out_ap, in_=ps,
                            func=mybir.ActivationFunctionType.Identity,
                            bias=b_sb, scale=1.0,
                        )
                    else:  # DVE eviction with bias
                        nc.vector.tensor_scalar_add(out_ap, ps, b_sb)
            # transpose VT into natural-layout v tiles (one transpose-mode region)
            for t in range(NT):
                pst = psT.tile([P, COLS], F16, tag="vtr", name=f"vtr{t}")
                nc.tensor.matmul(
                    pst,
                    vt_sb[:, t * P : (t + 1) * P],
                    ident[:COLS, :COLS],
                    is_transpose=True,
                )
                nc.vector.tensor_copy(
                    out=v_sb[:, t, :].rearrange("p (h c) -> p h c", h=HPC)[
                        :, :, 0:hd
                    ],
                    in_=pst.rearrange("p (h c) -> p h c", h=HPC),
                )

        # ---- phases B/C/D per batch, heads interleaved ----
        with (
            tc.tile_pool(name="expool", bufs=1) as expool,
            tc.tile_pool(name="apool", bufs=2) as apool,
            tc.tile_pool(name="spool", bufs=4) as spool,
            tc.tile_pool(name="opool", bufs=3) as opool,
            tc.tile_pool(name="psB", bufs=1, space="PSUM") as psB,
            tc.tile_pool(name="psC", bufs=2, space="PSUM") as psC,
            tc.tile_pool(name="psD", bufs=2, space="PSUM") as psD,
        ):
            for b in range(B):
                att_sb = apool.tile([COLS, S], F16, tag="att", name=f"att{b}")
                exps = [
                    expool.tile([P, expw], F16, tag=f"exp{h}", name=f"exp{b}_{h}")
                    for h in range(HPC)
                ]

                # -- B: scoresT -> exp, both heads' row-tiles in flight --
                for kt in range(SKT):
                    w = S - P * kt
                    off = offs[kt]
                    q0 = b * S + P * kt  # global q col of strip start
                    for p0 in range(0, w, PW):
                        pw_ = min(PW, w - p0)
                        pieces = [
                            psB.tile([P, PW], F32, tag=f"sc{h}",
                                     name=f"sc{b}_{kt}_{p0}_{h}")
                            for h in range(HPC)
                        ]
                        for s0 in range(0, pw_, 512):
                            sw = min(512, pw_ - s0)
                            for h in range(HPC):
                                hp = h * hd
                                nc.tensor.matmul(
                                    pieces[h][:, s0 : s0 + sw],
                                    kt_sb[hp : hp + hd,
                                          b * S + P * kt : b * S + P * (kt + 1)],
                                    qt_sb[hp : hp + hd,
                                          q0 + p0 + s0 : q0 + p0 + s0 + sw],
                                    start=True,
                                    stop=True,
                                )
                        for h in range(HPC):
                            nc.scalar.activation(
                                out=exps[h][:, off + p0 : off + p0 + pw_],
                                in_=pieces[h][:, 0:pw_],
                                func=mybir.ActivationFunctionType.Exp,
                            )
                    for h in range(HPC):
                        # causal mask on the diagonal block
                        nc.vector.tensor_tensor(
                            out=exps[h][:, off : off + P],
                            in0=exps[h][:, off : off + P],
                            in1=umask,
                            op=mybir.AluOpType.mult,
                        )

                # -- C: att_outT accumulation + normalize, heads interleaved --
                for c in range(NQC):
                    kt_hi = min(SKT, KPC * (c + 1))
                    pas = [
                        psC.tile([VTC, QCW], F32, tag="acc", name=f"acc{b}_{c}_{h}")
                        for h in range(HPC)
                    ]
                    for kt in range(kt_hi):
                        lo = max(QCW * c, P * kt)
                        n = QCW * (c + 1) - lo
                        if n <= 0:
                            continue
                        for h in range(HPC):
                            nc.tensor.matmul(
                                pas[h][:, lo - QCW * c : lo - QCW * c + n],
                                v_sb[:, b * SKT + kt, h * VTC : (h + 1) * VTC],
                                exps[h][:, offs[kt] + lo - P * kt :
                                        offs[kt] + lo - P * kt + n],
                                start=(kt == 0),
                                stop=(kt == kt_hi - 1),
                            )
                    for h in range(HPC):
                        hp = h * hd
                        recip = spool.tile([1, QCW], F32, tag="recip",
                                           name=f"recip{b}_{c}_{h}")
                        nc.vector.reciprocal_approx_fast(
                            out=recip, in_=pas[h][hd : hd + 1, :]
                        )
                        rbc = spool.tile([hd, QCW], F32, tag="rbc",
                                         name=f"rbc{b}_{c}_{h}")
                        nc.gpsimd.partition_broadcast(rbc, recip[0:1, :])
                        nc.vector.tensor_tensor(
                            out=att_sb[hp : hp + hd, QCW * c : QCW * (c + 1)],
                            in0=pas[h][0:hd, :],
                            in1=rbc,
                            op=mybir.AluOpType.mult,
                        )

                # -- D: output projection for batch b (column-tile pairs) --
                for m in range(cfg.D // P):
                    for c in range(NQC):
                        po = psD.tile([P, QCW], F32, tag="po", name=f"po{b}_{m}_{c}")
                        PH = P // 2
                        for half in range(2):
                            nc.tensor.matmul(
                                po[half * PH : (half + 1) * PH, :],
                                wo_sb[:, m * P + half * PH : m * P + (half + 1) * PH],
                                att_sb[:, QCW * c : QCW * (c + 1)],
                                start=True,
                                stop=True,
                                skip_group_check=True,
                            )
                        o_sb = opool.tile([P, QCW], F32, tag="o", name=f"o{b}_{m}_{c}")
                        if (m + c) % 2 == 0:
                            nc.vector.tensor_copy(out=o_sb, in_=po)
                        else:
                            nc.scalar.copy(out=o_sb, in_=po)
                        nc.sync.dma_start(
                            out=yt[
                                m * P : (m + 1) * P, b * S + QCW * c : b * S + QCW * (c + 1)
                            ],
                            in_=o_sb,
                        )


def build_model(cfg: Cfg) -> bass.Bass:
    nc = bacc.Bacc(
        "TRN2", target_bir_lowering=False, debug=False, enable_asserts=False
    )
    io = {
        "xt": nc.dram_tensor("xt", [cfg.D, cfg.ROWS], F16, kind="ExternalInput").ap(),
        "wq": nc.dram_tensor("wq", [cfg.D, cfg.COLS], F16, kind="ExternalInput").ap(),
        "wk": nc.dram_tensor("wk", [cfg.D, cfg.COLS], F16, kind="ExternalInput").ap(),
        "wv": nc.dram_tensor("wv", [cfg.D, cfg.COLS], F16, kind="ExternalInput").ap(),
        "wo": nc.dram_tensor("wo", [cfg.COLS, cfg.D], F16, kind="ExternalInput").ap(),
        "bq": nc.dram_tensor("bq", [cfg.COLS], F32, kind="ExternalInput").ap(),
        "bk": nc.dram_tensor("bk", [cfg.COLS], F32, kind="ExternalInput").ap(),
        "bv": nc.dram_tensor("bv", [cfg.COLS], F32, kind="ExternalInput").ap(),
        "yt": nc.dram_tensor("yt", [cfg.D, cfg.ROWS], F32, kind="ExternalOutput").ap(),
    }
    with tile.TileContext(nc) as tc:
        emit_attention(tc, io, cfg)
    nc.finalize()
    return nc


def shard_inputs(cfg: Cfg, x, Wq, bq, Wk, bk, Wv, bv, Wo, bo):
    """Full inputs -> per-core in_maps (host side)."""
    scale = 1.0 / np.sqrt(np.float32(cfg.hd))
    xt = np.ascontiguousarray(
        np.asarray(x, dtype=np.float32).reshape(cfg.ROWS, cfg.D).T
    ).astype(np.float16)
    in_maps = []
    for c in range(cfg.NCORES):
        sl = slice(cfg.COLS * c, cfg.COLS * (c + 1))
        in_maps.append(
            {
                "xt": xt,
                "wq": np.ascontiguousarray(np.asarray(Wq)[:, sl] * scale).astype(np.float16),
                "bq": np.ascontiguousarray(np.asarray(bq)[sl] * scale),
                "wk": np.ascontiguousarray(np.asarray(Wk)[:, sl]).astype(np.float16),
                "bk": np.ascontiguousarray(np.asarray(bk)[sl]),
                "wv": np.ascontiguousarray(np.asarray(Wv)[:, sl]).astype(np.float16),
                "bv": np.ascontiguousarray(np.asarray(bv)[sl]),
                "wo": np.ascontiguousarray(np.asarray(Wo)[sl, :]).astype(np.float16),
            }
        )
    return in_maps


def unshard_output(cfg: Cfg, per_core_yt, bo):
    acc = per_core_yt[0].astype(np.float32)
    for yt_c in per_core_yt[1:]:
        acc = acc + yt_c
    y = acc.T + np.asarray(bo, dtype=np.float32)
    return np.ascontiguousarray(y.reshape(cfg.B, cfg.S, cfg.D)).astype(np.float32)


_MODEL = None


def _get_model(cfg: Cfg):
    global _MODEL
    if _MODEL is None:
        _MODEL = build_model(cfg)
    return _MODEL


def kernel(x, Wq, bq, Wk, bk, Wv, bv, Wo, bo, _trace=False):
    from concourse.bass_utils import run_bass_kernel_spmd

    cfg = Cfg()
    in_maps = shard_inputs(cfg, x, Wq, bq, Wk, bk, Wv, bv, Wo, bo)
    nc = _get_model(cfg)
    res = run_bass_kernel_spmd(
        nc, in_maps, core_ids=list(range(cfg.NCORES)), trace=_trace
    )
    y = unshard_output(cfg, [r["yt"] for r in res.results], bo)
    if _trace:
        return y, res
    return y
